# revision 26
# baseline (speedup 1.0000x reference)
"""Trainium2 Bass kernel for GPT-style multi-head causal self-attention.

Problem shapes (hardcoded): B=2, S=2048, D=1024, H=16, HD=64.
  qkv = x @ c_attn_w + c_attn_b ; split q,k,v ; per-head causal softmax(q k^T/8) v ;
  merge heads ; out = a @ c_proj_w + c_proj_b.

Sharding over 8 NeuronCores (tensor parallel over heads + sequence parallel
for the output projection):
  - Each core owns 2 heads: it gets the matching 384 columns of c_attn_w
    (q/k/v slices) and computes Q^T, K^T, V for its heads over ALL rows.
    hidden_states is fed pre-transposed (X^T, bf16) and replicated.
  - Attention runs in "transposed score" form: S^T[k,q] = K^T.T @ Q^T so the
    softmax denominator comes from a ones-augmented V matmul and no P
    transposes are needed. Causal masking skips strictly-upper blocks,
    trims fully-masked leading columns of diagonal blocks, and multiplies
    exp() by a 0/1 tril band on the partial 128-wide diagonal band only.
  - An AllToAll converts head-sharding to row-sharding; each core computes
    its 512 rows of the output projection against the full c_proj_w.

v2 schedule: stage-1 (QKV projection) chunks are woven into the stage-2
attention loop so the Activation engine's exp work overlaps the PE's
projection matmuls (engine queues execute in program order). V is computed
rows-major directly (lhsT = X^T sub-tile) so no PE transposes / extra
copies are needed. Elementwise work is spread across ACT (Q/K bias-copies
via Identity), DVE (V bias-adds, normalize), and Pool/gpsimd (mask bands).
"""

import numpy as np

import concourse.bass as bass
import concourse.mybir as mybir
import concourse.tile as tile
from concourse import bacc
from concourse.bass_utils import run_bass_kernel_spmd

NCORES = 8
B, S, D, H = 2, 2048, 1024, 16
HD = D // H            # 64
HPC = H // NCORES      # 2 heads per core
DH = HPC * HD          # 128 local head dims
R = B * S              # 4096 rows
RPC = R // NCORES      # 512 rows per core
QT_TILE = 512          # q tile (moving free dim)
KC = 128               # k chunk (psum partitions)
NQT = S // QT_TILE     # 4 q-tiles per batch
DCH = D // 128         # 8 contraction chunks over D
NRC = R // QT_TILE     # 8 row chunks
PAIRW = 2              # k-chunks per exp tile

F32 = mybir.dt.float32
BF16 = mybir.dt.bfloat16
MMDT = mybir.dt.float32r  # score matmul tile dtype
EVDT = BF16               # E/V tile dtype (post-softmax matmul)

_CACHED_NC = None


def _tril_mask() -> np.ndarray:
    """mask[kl, ql] = 1 if kl <= ql else 0 — the in-band triangle, shared by
    every diagonal 128-block (global condition kl + 128*d <= 128*d + ql)."""
    import ml_dtypes
    kl = np.arange(KC)[:, None]
    ql = np.arange(KC)[None, :]
    return (kl <= ql).astype(np.float32).astype(ml_dtypes.bfloat16)


def _declare_io(nc):
    x_t = nc.dram_tensor("x_t", [D, R], BF16, kind="ExternalInput")
    w_qkv = nc.dram_tensor("w_qkv", [D, 3 * DH], BF16, kind="ExternalInput")
    b_qkv = nc.dram_tensor("b_qkv", [3 * DH], F32, kind="ExternalInput")
    w_p = nc.dram_tensor("w_p", [D, D], BF16, kind="ExternalInput")
    b_p = nc.dram_tensor("b_p", [D], F32, kind="ExternalInput")
    out = nc.dram_tensor("out", [RPC, D], F32, kind="ExternalOutput")
    masks_dram = nc.inline_tensor(_tril_mask(), name="tril_mask")
    return x_t, w_qkv, b_qkv, w_p, b_p, out, masks_dram


def build():
    nc = bacc.Bacc("TRN2", target_bir_lowering=False, debug=False,
                   num_devices=NCORES)
    io = _declare_io(nc)
    with tile.TileContext(nc) as tc:
        _build_body(nc, tc, *io)
    nc.compile()
    return nc


def _build_body(nc, tc, x_t, w_qkv, b_qkv, w_p, b_p, out, masks_dram,
                collectives=True):
    from contextlib import ExitStack
    ctx = ExitStack()
    with ctx:
        consts = ctx.enter_context(tc.tile_pool(name="consts", bufs=1))
        dram = ctx.enter_context(tc.tile_pool(name="dram", bufs=1, space="DRAM"))

        # ---- constants / weights to SBUF ----
        # issue order matters: x_c[0] + wqkv first (gate the first matmuls);
        # stage-3-only tensors (wp, bp) go last on the gpsimd queue.
        x_c = [consts.tile([128, DCH, QT_TILE], BF16, name=f"x_c{rc}")
               for rc in range(NRC)]

        def load_x(rc, splits=1):
            xsrc = (x_t.ap()[:, rc * QT_TILE:(rc + 1) * QT_TILE]
                    .rearrange("(c p) n -> p c n", p=128))
            step = DCH // splits
            for s in range(splits):
                dsl = slice(s * step, (s + 1) * step)
                nc.sync.dma_start(x_c[rc][:, dsl, :], xsrc[:, dsl, :])

        wqkv_sb = consts.tile([128, DCH, 3 * DH], BF16)
        nc.gpsimd.dma_start(wqkv_sb[:], w_qkv.ap().rearrange("(c p) n -> p c n", p=128))
        load_x(0, splits=4)
        bq_sb = consts.tile([128, 1], F32)
        nc.sync.dma_start(bq_sb[:], b_qkv.ap()[0:DH][:, None])
        bk_sb = consts.tile([128, 1], F32)
        nc.sync.dma_start(bk_sb[:], b_qkv.ap()[DH:2 * DH][:, None])
        bv_bcast = consts.tile([128, DH], BF16)
        nc.gpsimd.dma_start(bv_bcast[:],
                            b_qkv.ap()[2 * DH:3 * DH][None, :].to_broadcast([128, DH]))
        tril_sb = consts.tile([128, KC], BF16)
        nc.sync.dma_start(tril_sb[:], masks_dram.ap())
        ones64_sb = consts.tile([1, 64], BF16)
        nc.gpsimd.memset(ones64_sb[:], 1.0)
        for rc in range(1, NRC):
            load_x(rc)
        wp_sb = consts.tile([128, DCH, D], BF16)
        nc.gpsimd.dma_start(wp_sb[:], w_p.ap().rearrange("(c p) n -> p c n", p=128))
        bp_bcast = consts.tile([128, D], BF16)
        nc.gpsimd.dma_start(bp_bcast[:], b_p.ap()[None, :].to_broadcast([128, D]))

        # ---- persistent per-chunk Q^T/K^T/V tiles ----
        qkv_pool = ctx.enter_context(tc.tile_pool(name="qkv_pool", bufs=1))
        qt_c, kt_c, v_c = [], [], []
        for rc in range(NRC):
            qt_c.append(qkv_pool.tile([128, QT_TILE], MMDT, name=f"qt_{rc}"))
            kt_c.append(qkv_pool.tile([128, QT_TILE], MMDT, name=f"kt_{rc}"))
            # V rows-major per 128-row sub-tile: [v0(64) ones v1(64) ones]
            v_c.append(qkv_pool.tile([128, 4, 130], EVDT, name=f"v_{rc}"))

        # ---- stage pools (PSUM budget: s2 2x2 + av 2x1 + qk 1 + v 1 = 8) ----
        stage12 = ExitStack()
        qk_psum = stage12.enter_context(tc.tile_pool(name="qk_psum", bufs=1, space="PSUM"))
        v_psum = stage12.enter_context(tc.tile_pool(name="v_psum", bufs=1, space="PSUM"))
        att_psum = stage12.enter_context(tc.tile_pool(name="att_psum", bufs=2, space="PSUM"))
        pv_psum = stage12.enter_context(tc.tile_pool(name="pv_psum", bufs=2, space="PSUM"))
        epool = ctx.enter_context(tc.tile_pool(name="epool", bufs=4))
        npool = ctx.enter_context(tc.tile_pool(name="npool", bufs=3))
        ndram = ctx.enter_context(tc.tile_pool(name="ndram", bufs=3, space="DRAM"))

        cc_in = dram.tile([NCORES, DH, QT_TILE], BF16)
        cc_out = dram.tile([NCORES, DH, QT_TILE], BF16)

        def s1_q(rc):
            ps = qk_psum.tile([128, QT_TILE], F32, tag="qk_ps", name=f"qps_{rc}")
            for dc in range(DCH):
                nc.tensor.matmul(ps[:], wqkv_sb[:, dc, 0:DH], x_c[rc][:, dc, :],
                                 start=(dc == 0), stop=(dc == DCH - 1))
            nc.vector.tensor_scalar_add(qt_c[rc][:], ps[:], bq_sb[:])

        def s1_k(rc):
            ps = qk_psum.tile([128, QT_TILE], F32, tag="qk_ps", name=f"kps_{rc}")
            for dc in range(DCH):
                nc.tensor.matmul(ps[:], wqkv_sb[:, dc, DH:2 * DH], x_c[rc][:, dc, :],
                                 start=(dc == 0), stop=(dc == DCH - 1))
            nc.vector.tensor_scalar_add(kt_c[rc][:], ps[:], bk_sb[:])

        def s1_v(rc, sts):
            # rows-major V: out[q rows, v dims] per 128-row sub-tile
            nc.gpsimd.memset(v_c[rc][:, sts[0]:sts[-1] + 1, :], 1.0)
            ps = v_psum.tile([128, len(sts), DH], F32, tag="v_ps",
                             name=f"vps_{rc}_{sts[0]}")
            for i, st in enumerate(sts):
                for dc in range(DCH):
                    nc.tensor.matmul(
                        ps[:, i, :],
                        x_c[rc][:, dc, st * 128:(st + 1) * 128],
                        wqkv_sb[:, dc, 2 * DH:3 * DH],
                        start=(dc == 0), stop=(dc == DCH - 1))
            for i, st in enumerate(sts):
                nc.vector.tensor_tensor(
                    v_c[rc][:, st, 0:64], ps[:, i, 0:64], bv_bcast[:, 0:64],
                    mybir.AluOpType.add)
                nc.vector.tensor_tensor(
                    v_c[rc][:, st, 65:129], ps[:, i, 64:128], bv_bcast[:, 64:128],
                    mybir.AluOpType.add)

        # Weave plan: Q(g) must land before q-tile g starts (it is g's q
        # tile); K(g)/V(g) are only read by q-tile g's diagonal blocks (last
        # two j's), so they weave into q-tile g itself at early j's. Batch
        # boundaries: chunk 4's Q/K/V all weave into (0,3) since (1,0) is
        # all-diagonal. Entries: qtile index t -> [(emit_at_j, piece)].
        def P(kind, g):
            if kind == "q":
                return lambda: s1_q(g)
            if kind == "k":
                return lambda: s1_k(g)
            if kind == "v01":
                return lambda: s1_v(g, (0, 1))
            return lambda: s1_v(g, (2, 3))

        weave = {
            t: [(j, P(kind, t + 1)) for j, kind in
                enumerate(("q", "k", "v01", "v23"))]
            for t in range(NRC - 1)
        }
        weave[NRC - 1] = []

        # prologue: first row chunk fully before the first q-tile
        s1_q(0)
        s1_k(0)
        s1_v(0, (0, 1))
        s1_v(0, (2, 3))

        # ---- stage 2 with stage-1 weaving ----
        for b in range(B):
            for qq in range(NQT):
                dest = b * NQT + qq
                rcq = b * NQT + qq
                nkk = 4 * qq + 4
                npairs = nkk // PAIRW
                pieces = list(weave[dest])

                av_ps = [pv_psum.tile([128, QT_TILE], F32, tag="av_ps",
                                      name=f"av_ps_{b}_{qq}_{h}")
                         for h in range(HPC)]
                for j in range(npairs):
                    s2 = [att_psum.tile([128, PAIRW * QT_TILE], F32, tag="s2",
                                        name=f"s2_{b}_{qq}_{j}_{h}")
                          for h in range(HPC)]
                    # interleave heads so consecutive matmuls hit different
                    # PE row-groups (K=64 at partitions 0/64 run concurrently)
                    for ki in range(PAIRW):
                        kk = PAIRW * j + ki
                        rck = b * NQT + kk // 4
                        ko = (kk % 4) * KC
                        d = kk - 4 * qq
                        mo = KC * d if d > 0 else 0  # trim fully-masked q cols
                        for h in range(HPC):
                            hsl = slice(h * HD, (h + 1) * HD)
                            nc.tensor.matmul(
                                s2[h][:, ki * QT_TILE + mo:(ki + 1) * QT_TILE],
                                kt_c[rck][hsl, ko:ko + KC],
                                qt_c[rcq][hsl, mo:],
                                start=True, stop=True)
                    while pieces and pieces[0][0] <= j:
                        pieces.pop(0)[1]()    # weave stage-1 QKV work
                    e_sb = [None, None]
                    for h in range(HPC):
                        et = epool.tile([128, PAIRW * QT_TILE], EVDT, tag="e_sb",
                                        name=f"e_{b}_{qq}_{j}_{h}")
                        kks = [PAIRW * j + ki for ki in range(PAIRW)]
                        ds = [kk - 4 * qq for kk in kks]
                        if ds[-1] < 0:
                            # strictly below diagonal: one full-width exp
                            nc.scalar.activation(
                                et[:], s2[h][:],
                                mybir.ActivationFunctionType.Exp, scale=0.125)
                        else:
                            for ki in range(PAIRW):
                                mo = KC * ds[ki] if ds[ki] > 0 else 0
                                nc.scalar.activation(
                                    et[:, ki * QT_TILE + mo:(ki + 1) * QT_TILE],
                                    s2[h][:, ki * QT_TILE + mo:(ki + 1) * QT_TILE],
                                    mybir.ActivationFunctionType.Exp, scale=0.125)
                            for ki in range(PAIRW):
                                if ds[ki] >= 0:  # zero the in-band triangle
                                    mo = KC * ds[ki]
                                    nc.gpsimd.tensor_tensor(
                                        et[:, ki * QT_TILE + mo:ki * QT_TILE + mo + KC],
                                        et[:, ki * QT_TILE + mo:ki * QT_TILE + mo + KC],
                                        tril_sb[:], mybir.AluOpType.mult)
                        e_sb[h] = et
                    for h in range(HPC):
                        vcols = slice(0, 65) if h == 0 else slice(65, 130)
                        for ki in range(PAIRW):
                            kk = PAIRW * j + ki
                            rck = b * NQT + kk // 4
                            d = kk - 4 * qq
                            po = KC * d if d > 0 else 0
                            nc.tensor.matmul(
                                av_ps[h][0:65, po:QT_TILE],
                                v_c[rck][:, kk % 4, vcols],
                                e_sb[h][:, ki * QT_TILE + po:(ki + 1) * QT_TILE],
                                start=(kk == 0), stop=(kk == nkk - 1))
                while pieces:
                    pieces.pop(0)[1]()
                # normalize: a = pv * (1/sumexp)  (v-bias already folded into V)
                # copy PSUM out promptly (two partition-0-based copies; a
                # single [65,...] copy + recip at base partition 64
                # miscomputes on HW) so av_ps frees for the next q-tile;
                # the recip/broadcast chain then runs on SBUF off the
                # critical path (bf16 broadcast via the gpsimd DMA queue).
                # normalize: a = pv * (1/sumexp). sumexp sits at psum partition
                # 64; recip runs at partition 0 in SBUF (recip at psum base 64
                # miscomputes on HW), then a rank-1 PE matmul (ones x rec)
                # broadcasts 1/sumexp into the tile's free partitions 64:128
                # and one DVE multiply produces the bf16 a-tile. No DMA hops.
                for h in range(HPC):
                    se_sb = npool.tile([1, QT_TILE], F32, tag="se_sb")
                    nc.vector.tensor_copy(se_sb[:], av_ps[h][64:65, :])
                    au_sb = npool.tile([64, QT_TILE], BF16, tag="au_sb",
                                       name=f"au_{b}_{qq}_{h}")
                    nc.vector.tensor_copy(au_sb[:], av_ps[h][0:64, :])
                    rec_sb = npool.tile([1, QT_TILE], F32, tag="rec_sb")
                    nc.vector.reciprocal_approx_fast(rec_sb[:], se_sb[:])
                    recb_sb = npool.tile([1, QT_TILE], BF16, tag="recb_sb")
                    nc.vector.tensor_copy(recb_sb[:], rec_sb[:])
                    nc.tensor.matmul(av_ps[h][64:128, :], ones64_sb[:],
                                     recb_sb[:], start=True, stop=True)
                    a_sb = npool.tile([64, QT_TILE], BF16, tag="a_sb")
                    nc.vector.tensor_tensor(
                        a_sb[:], au_sb[:], av_ps[h][64:128, :],
                        mybir.AluOpType.mult)
                    nc.sync.dma_start(cc_in[dest, h * HD:(h + 1) * HD, :], a_sb[:])

        if collectives:
            nc.gpsimd.collective_compute(
                "AllToAll", mybir.AluOpType.bypass,
                replica_groups=[list(range(NCORES))],
                ins=[cc_in.opt()], outs=[cc_out.opt()],
            )
        else:
            for r in range(NCORES):
                nc.sync.dma_start(cc_out[r], cc_in[r])

        # ---- stage 3: output projection on own 512 rows ----
        stage12.close()
        opool = ctx.enter_context(tc.tile_pool(name="opool", bufs=2))
        opsum = ctx.enter_context(tc.tile_pool(name="opsum", bufs=8, space="PSUM"))
        o_ps = [opsum.tile([128, QT_TILE], F32, tag="o_ps", name=f"o_ps_{i}")
                for i in range(8)]
        oqueues = [nc.sync, nc.scalar]
        ach = [opool.tile([128, QT_TILE], BF16, name=f"ach_{kc}")
               for kc in range(NCORES)]
        for kc in range(NCORES):              # contraction chunk = source core
            oqueues[kc % 2].dma_start(ach[kc][:], cc_out[kc])
        # tile-major so each output tile drains while later tiles compute
        for mt in range(RPC // 128):          # 4 row tiles
            for nt in range(D // QT_TILE):    # 2 dout tiles
                i = mt * 2 + nt
                for kc in range(NCORES):
                    nc.tensor.matmul(
                        o_ps[i][:], ach[kc][:, mt * 128:(mt + 1) * 128],
                        wp_sb[:, kc, nt * QT_TILE:(nt + 1) * QT_TILE],
                        start=(kc == 0), stop=(kc == NCORES - 1))
                o_sb = opool.tile([128, QT_TILE], F32, tag="o_sb")
                nc.vector.tensor_tensor(
                    o_sb[:], o_ps[i][:],
                    bp_bcast[:, nt * QT_TILE:(nt + 1) * QT_TILE],
                    mybir.AluOpType.add)
                oqueues[i % 2].dma_start(
                    out.ap()[mt * 128:(mt + 1) * 128,
                             nt * QT_TILE:(nt + 1) * QT_TILE], o_sb[:])


def _shard_inputs(hidden_states, c_attn_w, c_attn_b, c_proj_w, c_proj_b):
    import ml_dtypes
    X = np.asarray(hidden_states, dtype=np.float32).reshape(R, D)
    x_t = np.ascontiguousarray(X.T).astype(ml_dtypes.bfloat16)
    w_p = np.ascontiguousarray(c_proj_w).astype(ml_dtypes.bfloat16)
    b_p = np.ascontiguousarray(c_proj_b).astype(np.float32)
    in_maps = []
    for c in range(NCORES):
        cols = []
        for part in range(3):
            lo = part * D + c * DH
            cols.append(np.arange(lo, lo + DH))
        cols = np.concatenate(cols)
        in_maps.append({
            "x_t": x_t,
            "w_qkv": np.ascontiguousarray(c_attn_w[:, cols]).astype(ml_dtypes.bfloat16),
            "b_qkv": np.ascontiguousarray(c_attn_b[cols]).astype(np.float32),
            "w_p": w_p,
            "b_p": b_p,
        })
    return in_maps


def _get_nc():
    global _CACHED_NC
    if _CACHED_NC is None:
        _CACHED_NC = build()
    return _CACHED_NC


def kernel(hidden_states, c_attn_w, c_attn_b, c_proj_w, c_proj_b):
    nc = _get_nc()
    in_maps = _shard_inputs(hidden_states, c_attn_w, c_attn_b,
                            c_proj_w, c_proj_b)
    res = run_bass_kernel_spmd(nc, in_maps, core_ids=list(range(NCORES)))
    full = np.concatenate([res.results[c]["out"] for c in range(NCORES)], axis=0)
    return full.reshape(B, S, D).astype(np.float32)
